# revision 12
# baseline (speedup 1.0000x reference)
"""CenterLoss kernel (v15: v14 fused DVE tail + v12 ACT per-j normalize
(no DVE 2-port op overlapping Q7 descgen): final chunk squares+accumulates on DVE via
scalar_tensor_tensor, skipping the last cross-engine hop: bf16 gathered centers via SWDGE cast-in-DMA,
bf16 nx -> 2x DVE sub and 2x ACT square rates, halved gather drain bytes) for Trainium2 (8 NeuronCores, data-parallel over batch).

loss = mean_i( ||nx_i - c_{l_i}||^2 ),  nx_i = x_i / max(||x_i||, EPS)

Per core (2048 rows; row p*16+j at SBUF partition p, free block j):
  - gather centers[labels] with 16 INDIRECT1D ops (HW consumes one offset
    per dest partition; ~1.35us/op Q7 descgen is the kernel's floor).
  - x pipeline shadowed under the gathers: square (ACT), rowsum (DVE),
    max+recip (DVE), sqrt (ACT), nx = x * inv (DVE). A dummy Sqrt is the
    first ACT op so ONE act-table load (sqrt_and_others, which also has
    Square) happens early, overlapped with the label DMA.
  - tail per chunk: DVE d = nx - c in place, ACT Square-accumulates d^2.
    Chunks [6,6,3,1]: the last chunk is 1 op so only 128 rows trail.
Host combines: loss = sum(out) / B.
"""

import numpy as np

B, C, D = 16384, 8192, 64
N_CORES = 8
ROWS = B // N_CORES
P = 128
J = ROWS // P            # 16
F = J * D
CHUNKS = [6, 6, 3, 1]    # j-blocks per compute chunk (last is 1 op's worth)
assert sum(CHUNKS) == J
NB = len(CHUNKS)
CSTART = [sum(CHUNKS[:b]) for b in range(NB)]
CUM = [sum(CHUNKS[:b + 1]) for b in range(NB)]

_CACHE = {}


def _build():
    from contextlib import ExitStack

    import concourse.bass as bass
    from concourse import bacc, mybir

    nc = bacc.Bacc("TRN2", target_bir_lowering=False, debug=False,
                   num_devices=N_CORES, dynamic_dma_scratch_size=65536)
    f32 = mybir.dt.float32
    bf16 = mybir.dt.bfloat16
    x = nc.dram_tensor("x", [ROWS, D], bf16, kind="ExternalInput").ap()
    labels = nc.dram_tensor("labels", [P, J], mybir.dt.int32,
                            kind="ExternalInput").ap()
    centers = nc.dram_tensor("centers", [C, D], f32,
                             kind="ExternalInput").ap()
    out = nc.dram_tensor("out", [P, NB], f32, kind="ExternalOutput").ap()

    with ExitStack() as ctx:
        def sb(n, s, dt=f32):
            return ctx.enter_context(nc.sbuf_tensor(n, s, dt))
        lab_t = sb("lab_t", [P, J], mybir.dt.int32)
        x_t = sb("x_t", [P, F], bf16)
        c_t = sb("c_t", [P, F], bf16)
        d_t = sb("d_t", [P, F], bf16)
        xx = sb("xx", [P, F])
        sx = sb("sx", [P, J])
        rcp = sb("rcp", [P, J])
        inv = sb("inv", [P, J])
        nx = sb("nx", [P, F], bf16)
        dum = sb("dum", [P, 1])
        acc = sb("acc", [P, NB])
        L = ctx.enter_context(nc.semaphore("Lsem"))
        X = ctx.enter_context(nc.semaphore("Xsem"))
        G = [ctx.enter_context(nc.semaphore(f"G{i}")) for i in range(NB)]
        A = ctx.enter_context(nc.semaphore("Asem"))   # ACT-produced events
        V = ctx.enter_context(nc.semaphore("Vsem"))   # DVE-produced events

        # ---- Sync: labels in, result out ----
        nc.sync.dma_start(lab_t[:], labels[:]).then_inc(L, 16)
        # x AFTER labels on the same queue: the 8KB labels transfer is
        # latency-critical (gates all 16 gathers) and must not share SDMA
        # round-robin with the big x transfer.
        nc.sync.dma_start(x_t[:], x.rearrange("(p j) d -> p (j d)", p=P)
                          ).then_inc(X, 16)
        nc.sync.wait_ge(A, 2 + NB)
        nc.sync.wait_ge(V, 4 + NB)
        nc.sync.dma_start(out, acc[:]).then_inc(L, 16)
        nc.sync.wait_ge(L, 32)

        # ---- GpSimd: 16 indirect gathers back to back ----
        nc.gpsimd.wait_ge(L, 16)
        for j in range(J):
            b = next(i for i in range(NB) if CSTART[i] <= j < CUM[i])
            nc.gpsimd.indirect_dma_start(
                out=c_t[:, j * D:(j + 1) * D],
                out_offset=None,
                in_=centers[:],
                in_offset=bass.IndirectOffsetOnAxis(ap=lab_t[:, j:j + 1],
                                                    axis=0),
            ).then_inc(G[b], 16)

        # ---- Scalar/ACT ----
        # A events: 1=xx, 2=inv(sqrt), 2+b+1 = chunk b accumulated
        # dummy: forces the single act-table load (sqrt_and_others) early
        nc.scalar.sqrt(dum[:], nc.const_aps.scalar_like(1.0, dum[:]))
        nc.scalar.wait_ge(X, 16)
        nc.scalar.square(xx[:], x_t[:]).then_inc(A, 1)
        nc.scalar.wait_ge(V, 3)
        nc.scalar.sqrt(inv[:], rcp[:]).then_inc(A, 1)
        nc.scalar.wait_ge(A, 2)
        for j in range(J):
            inst = nc.scalar.activation(
                nx[:, j * D:(j + 1) * D], x_t[:, j * D:(j + 1) * D],
                mybir.ActivationFunctionType.Copy, bias=0.0,
                scale=inv[:, j:j + 1])
            if j == J - 1:
                inst.then_inc(A, 1)
        for b in range(NB - 1):
            f0, f1 = CSTART[b] * D, CUM[b] * D
            nc.scalar.wait_ge(V, 4 + b)
            nc.scalar.activation(d_t[:, f0:f1], d_t[:, f0:f1],
                                 mybir.ActivationFunctionType.Square,
                                 accum_out=acc[:, b:b + 1]).then_inc(A, 1)

        # ---- Vector/DVE ----
        # V events: 1=sx, 2=max, 3=rcp, 4=nx, 4+b+1 = chunk b sub done
        nc.vector.wait_ge(A, 1)
        nc.vector.reduce_sum(sx[:], xx[:].rearrange("p (j d) -> p j d", d=D),
                             axis=mybir.AxisListType.X).then_inc(V, 1)
        nc.vector.wait_ge(V, 1)
        nc.vector.tensor_scalar_max(sx[:], sx[:], 1e-24).then_inc(V, 1)
        nc.vector.wait_ge(V, 2)
        nc.vector.reciprocal(rcp[:], sx[:]).then_inc(V, 1)
        nc.vector.wait_ge(A, 3)
        for b in range(NB):
            f0, f1 = CSTART[b] * D, CUM[b] * D
            nc.vector.wait_ge(G[b], 16 * CHUNKS[b])
            nc.vector.tensor_sub(d_t[:, f0:f1], nx[:, f0:f1],
                                 c_t[:, f0:f1]).then_inc(V, 1)
        fL0, fL1 = CSTART[NB - 1] * D, CUM[NB - 1] * D
        nc.vector.wait_ge(V, 3 + NB)
        nc.vector.scalar_tensor_tensor(
            out=c_t[:, fL0:fL1], in0=d_t[:, fL0:fL1], scalar=1.0,
            in1=d_t[:, fL0:fL1], op0=mybir.AluOpType.mult,
            op1=mybir.AluOpType.mult,
            accum_out=acc[:, NB - 1:NB]).then_inc(V, 1)

    nc.compile()
    return nc


def _get_nc():
    if "nc" not in _CACHE:
        _CACHE["nc"] = _build()
    return _CACHE["nc"]


def _in_map(np_bf16, x_shard, labels_shard, centers):
    return {
        "x": np.ascontiguousarray(np.asarray(x_shard).astype(np_bf16)),
        "labels": np.ascontiguousarray(
            np.asarray(labels_shard).astype(np.int32).reshape(P, J)),
        "centers": centers,
    }


def _run(x, labels, centers, trace=False):
    from concourse import mybir
    from concourse.bass_utils import run_bass_kernel_spmd

    np_bf16 = mybir.dt.np(mybir.dt.bfloat16)

    x = np.ascontiguousarray(np.asarray(x, dtype=np.float32))
    labels = np.asarray(labels).astype(np.int32)
    centers = np.ascontiguousarray(np.asarray(centers, dtype=np.float32))

    in_maps = [_in_map(np_bf16, x[i * ROWS:(i + 1) * ROWS],
                       labels[i * ROWS:(i + 1) * ROWS], centers)
               for i in range(N_CORES)]
    res = run_bass_kernel_spmd(_get_nc(), in_maps,
                               core_ids=list(range(N_CORES)), trace=trace)
    total = np.float64(0.0)
    for r in res.results:
        total += np.float64(r["out"].sum(dtype=np.float64))
    loss = np.array(np.float32(total / B))
    return loss, res


def kernel(x, labels, centers):
    loss, _ = _run(x, labels, centers, trace=False)
    return loss



# revision 13
# speedup vs baseline: 1.0362x; 1.0362x over previous
"""CenterLoss kernel (v15: v14 fused DVE tail + v12 ACT per-j normalize
(no DVE 2-port op overlapping Q7 descgen): final chunk squares+accumulates on DVE via
scalar_tensor_tensor, skipping the last cross-engine hop: bf16 gathered centers via SWDGE cast-in-DMA,
bf16 nx -> 2x DVE sub and 2x ACT square rates, halved gather drain bytes) for Trainium2 (8 NeuronCores, data-parallel over batch).

loss = mean_i( ||nx_i - c_{l_i}||^2 ),  nx_i = x_i / max(||x_i||, EPS)

Per core (2048 rows; row p*16+j at SBUF partition p, free block j):
  - gather centers[labels] with 16 INDIRECT1D ops (HW consumes one offset
    per dest partition; ~1.35us/op Q7 descgen is the kernel's floor).
  - x pipeline shadowed under the gathers: square (ACT), rowsum (DVE),
    max+recip (DVE), sqrt (ACT), nx = x * inv (DVE). A dummy Sqrt is the
    first ACT op so ONE act-table load (sqrt_and_others, which also has
    Square) happens early, overlapped with the label DMA.
  - tail per chunk: DVE d = nx - c in place, ACT Square-accumulates d^2.
    Chunks [6,6,3,1]: the last chunk is 1 op so only 128 rows trail.
Host combines: loss = sum(out) / B.
"""

import numpy as np

B, C, D = 16384, 8192, 64
N_CORES = 8
ROWS = B // N_CORES
P = 128
J = ROWS // P            # 16
F = J * D
CHUNKS = [6, 6, 3, 1]    # j-blocks per compute chunk (last is 1 op's worth)
assert sum(CHUNKS) == J
NB = len(CHUNKS)
CSTART = [sum(CHUNKS[:b]) for b in range(NB)]
CUM = [sum(CHUNKS[:b + 1]) for b in range(NB)]

_CACHE = {}


def _build():
    from contextlib import ExitStack

    import concourse.bass as bass
    from concourse import bacc, mybir

    nc = bacc.Bacc("TRN2", target_bir_lowering=False, debug=False,
                   num_devices=N_CORES, dynamic_dma_scratch_size=65536)
    f32 = mybir.dt.float32
    bf16 = mybir.dt.bfloat16
    x = nc.dram_tensor("x", [ROWS, D], bf16, kind="ExternalInput").ap()
    labels = nc.dram_tensor("labels", [P, J], mybir.dt.int32,
                            kind="ExternalInput").ap()
    centers = nc.dram_tensor("centers", [C, D], f32,
                             kind="ExternalInput").ap()
    out = nc.dram_tensor("out", [P, NB], f32, kind="ExternalOutput").ap()

    with ExitStack() as ctx:
        def sb(n, s, dt=f32):
            return ctx.enter_context(nc.sbuf_tensor(n, s, dt))
        lab_t = sb("lab_t", [P, J], mybir.dt.int32)
        x_t = sb("x_t", [P, F], bf16)
        c_t = sb("c_t", [P, F], bf16)
        d_t = sb("d_t", [P, F], bf16)
        xx = sb("xx", [P, F])
        sx = sb("sx", [P, J])
        rcp = sb("rcp", [P, J])
        inv = sb("inv", [P, J])
        nx = sb("nx", [P, F], bf16)
        dum = sb("dum", [P, 1])
        acc = sb("acc", [P, NB])
        L = ctx.enter_context(nc.semaphore("Lsem"))
        X = ctx.enter_context(nc.semaphore("Xsem"))
        G = [ctx.enter_context(nc.semaphore(f"G{i}")) for i in range(NB)]
        A = ctx.enter_context(nc.semaphore("Asem"))   # ACT-produced events
        V = ctx.enter_context(nc.semaphore("Vsem"))   # DVE-produced events

        # ---- Sync: labels in, result out ----
        nc.sync.dma_start(lab_t[:], labels[:]).then_inc(L, 16)
        # x AFTER labels on the same queue: the 8KB labels transfer is
        # latency-critical (gates all 16 gathers) and must not share SDMA
        # round-robin with the big x transfer.
        nc.sync.dma_start(x_t[:], x.rearrange("(p j) d -> p (j d)", p=P)
                          ).then_inc(X, 16)
        nc.sync.wait_ge(A, 2 + NB)
        nc.sync.wait_ge(V, 4 + NB)
        # No wait on the out-DMA completion sem: the NRT postamble's
        # sync_barrier + dma_rearm drains the rings before teardown, so the
        # ~1us HBM write receipt overlaps the postamble instead of gating it.
        nc.sync.dma_start(out, acc[:]).then_inc(L, 16)

        # ---- GpSimd: 16 indirect gathers back to back ----
        nc.gpsimd.wait_ge(L, 16)
        for j in range(J):
            b = next(i for i in range(NB) if CSTART[i] <= j < CUM[i])
            nc.gpsimd.indirect_dma_start(
                out=c_t[:, j * D:(j + 1) * D],
                out_offset=None,
                in_=centers[:],
                in_offset=bass.IndirectOffsetOnAxis(ap=lab_t[:, j:j + 1],
                                                    axis=0),
            ).then_inc(G[b], 16)

        # ---- Scalar/ACT ----
        # A events: 1=xx, 2=inv(sqrt), 2+b+1 = chunk b accumulated
        # dummy: forces the single act-table load (sqrt_and_others) early
        nc.scalar.sqrt(dum[:], nc.const_aps.scalar_like(1.0, dum[:]))
        nc.scalar.wait_ge(X, 16)
        nc.scalar.square(xx[:], x_t[:]).then_inc(A, 1)
        nc.scalar.wait_ge(V, 3)
        nc.scalar.sqrt(inv[:], rcp[:]).then_inc(A, 1)
        nc.scalar.wait_ge(A, 2)
        for j in range(J):
            inst = nc.scalar.activation(
                nx[:, j * D:(j + 1) * D], x_t[:, j * D:(j + 1) * D],
                mybir.ActivationFunctionType.Copy, bias=0.0,
                scale=inv[:, j:j + 1])
            if j == J - 1:
                inst.then_inc(A, 1)
        for b in range(NB - 1):
            f0, f1 = CSTART[b] * D, CUM[b] * D
            nc.scalar.wait_ge(V, 4 + b)
            nc.scalar.activation(d_t[:, f0:f1], d_t[:, f0:f1],
                                 mybir.ActivationFunctionType.Square,
                                 accum_out=acc[:, b:b + 1]).then_inc(A, 1)

        # ---- Vector/DVE ----
        # V events: 1=sx, 2=max, 3=rcp, 4=nx, 4+b+1 = chunk b sub done
        nc.vector.wait_ge(A, 1)
        nc.vector.reduce_sum(sx[:], xx[:].rearrange("p (j d) -> p j d", d=D),
                             axis=mybir.AxisListType.X).then_inc(V, 1)
        nc.vector.wait_ge(V, 1)
        nc.vector.tensor_scalar_max(sx[:], sx[:], 1e-24).then_inc(V, 1)
        nc.vector.wait_ge(V, 2)
        nc.vector.reciprocal(rcp[:], sx[:]).then_inc(V, 1)
        nc.vector.wait_ge(A, 3)
        for b in range(NB):
            f0, f1 = CSTART[b] * D, CUM[b] * D
            nc.vector.wait_ge(G[b], 16 * CHUNKS[b])
            nc.vector.tensor_sub(d_t[:, f0:f1], nx[:, f0:f1],
                                 c_t[:, f0:f1]).then_inc(V, 1)
        fL0, fL1 = CSTART[NB - 1] * D, CUM[NB - 1] * D
        nc.vector.wait_ge(V, 3 + NB)
        nc.vector.scalar_tensor_tensor(
            out=c_t[:, fL0:fL1], in0=d_t[:, fL0:fL1], scalar=1.0,
            in1=d_t[:, fL0:fL1], op0=mybir.AluOpType.mult,
            op1=mybir.AluOpType.mult,
            accum_out=acc[:, NB - 1:NB]).then_inc(V, 1)

    nc.compile()
    return nc


def _get_nc():
    if "nc" not in _CACHE:
        _CACHE["nc"] = _build()
    return _CACHE["nc"]


def _in_map(np_bf16, x_shard, labels_shard, centers):
    return {
        "x": np.ascontiguousarray(np.asarray(x_shard).astype(np_bf16)),
        "labels": np.ascontiguousarray(
            np.asarray(labels_shard).astype(np.int32).reshape(P, J)),
        "centers": centers,
    }


def _run(x, labels, centers, trace=False):
    from concourse import mybir
    from concourse.bass_utils import run_bass_kernel_spmd

    np_bf16 = mybir.dt.np(mybir.dt.bfloat16)

    x = np.ascontiguousarray(np.asarray(x, dtype=np.float32))
    labels = np.asarray(labels).astype(np.int32)
    centers = np.ascontiguousarray(np.asarray(centers, dtype=np.float32))

    in_maps = [_in_map(np_bf16, x[i * ROWS:(i + 1) * ROWS],
                       labels[i * ROWS:(i + 1) * ROWS], centers)
               for i in range(N_CORES)]
    res = run_bass_kernel_spmd(_get_nc(), in_maps,
                               core_ids=list(range(N_CORES)), trace=trace)
    total = np.float64(0.0)
    for r in res.results:
        total += np.float64(r["out"].sum(dtype=np.float64))
    loss = np.array(np.float32(total / B))
    return loss, res


def kernel(x, labels, centers):
    loss, _ = _run(x, labels, centers, trace=False)
    return loss

